# revision 54
# baseline (speedup 1.0000x reference)
"""Trainium2 Bass kernel for nn_BackpropKalmanFilter (v4).

After the Riccati recursion converges the filter is the LTI recursion
    x_t = A x_{t-1} + K z_t,   A = (I - K H) F   (rho(A) ~ 0.96)
Blocks of L=32 steps, U_b = C Z_b (end-of-block response), halo over J=4
previous blocks; the pre-convergence transient is recomputed on the host.

Structure (vs v2; each driven by a trace finding):
  - 4 stripes [508, 508, 504, 440] processed as two PAIRS, mt-phase-major:
    per phase both stripes' Z matmuls share each stationary; each phase's
    halo+eviction is software-pipelined one phase later (emitted after the
    NEXT phase's Z matmuls, carried across the pair boundary too) so halo
    dependencies have extra slack and the next weight loads prefetch under
    the halo matmuls.
  - z input resident in SBUF, loaded as few LONG-descriptor DMAs (3928 B
    rows, split only where early completion matters): DMA queues are
    packet-rate bound (~135 pkts/us/queue), not byte bound.
  - All SBUF tiles are singles (no pool rotation); per-queue FIFO order
    paces input arrival in consumption order; outputs kick per stripe,
    sync queue for mt01, scalar/sync mix for mt23 (never a scalar-engine
    kick mid-compute: it stalls the ACT copy pipeline -> PSUM WAR -> PE).
  - 8+8 HAM warmup matmuls (coarse+fine tail) keep the PE busy until z
    lands: HAM needs
    ~3.4us of sustained activity to unthrottle 1.2->2.4 GHz and any
    multi-us idle re-throttles it.
All operands bf16, accumulation f32, output bf16.
"""
import os
import sys

import numpy as np

sys.path.insert(0, "/opt/trn_rl_repo")
sys.path.insert(0, "/root/.axon_site")
sys.path.insert(0, "/root/.axon_site/_ro/pypackages")

N, M = 16, 8          # state / measurement dims
T = 500_000
L = 32                # block length
J = 4                 # halo blocks (D = J*L = 128 decay length)
NCORES = 8
KB = 1960                         # blocks per core
TTOT = NCORES * KB * L            # 501760 padded steps
KC = KB + J                       # columns incl. halo
DTYPE_MODE = os.environ.get("KAL_DTYPE", "bf16")   # bf16 | fp32r | fp32
OUT_MODE = os.environ.get("KAL_OUTDT", "bf16")     # bf16 | fp32

WIDTHS = [508, 508, 504, 440]
STARTS = [0]
for _w in WIDTHS[:-1]:
    STARTS.append(STARTS[-1] + _w)
NS = len(WIDTHS)
# output groups: (first stripe idx, n stripes)
YGROUPS = [(i, 1) for i in range(NS)]

_cache = {}


# ----------------------------------------------------------------- host math
def _riccati(F, H, Q, R):
    F64, H64 = F.astype(np.float64), H.astype(np.float64)
    Q64, R64 = Q.astype(np.float64), R.astype(np.float64)
    P = np.eye(N)
    prevK = None
    T1 = None
    for t in range(2048):
        P = F64 @ P @ F64.T + Q64
        S = H64 @ P @ H64.T + R64
        K = P @ H64.T @ np.linalg.inv(S)
        P = (np.eye(N) - K @ H64) @ P
        if prevK is not None and T1 is None and np.abs(K - prevK).max() < 1e-13:
            T1 = t
        prevK = K
    assert T1 is not None
    A = (np.eye(N) - K @ H64) @ F64
    return K, A, T1


def _build_weights(F, H, Q, R):
    """Returns (wT [128,2,512], ct [128,2,128], s01 [128,512], T0) in f64."""
    K_ss, A, T1 = _riccati(F, H, Q, R)
    npow = L * J + 2
    Apow = np.empty((npow, N, N))
    Apow[0] = np.eye(N)
    for i in range(1, npow):
        Apow[i] = Apow[i - 1] @ A
    AK = Apow @ K_ss                                   # A^d K  (16 x 8)

    C = np.concatenate([AK[L - 1 - j] for j in range(L)], axis=1)  # (16, 256)
    G = np.zeros((N * L, M * L))
    for i in range(L):
        for j in range(i + 1):
            G[i * N:(i + 1) * N, j * M:(j + 1) * M] = AK[i - j]
    Pm = np.concatenate([Apow[i + 1] for i in range(L)], axis=0)   # (512, 16)

    wT = np.empty((128, 2, 512))
    for i in range(2):
        wT[:, i, :] = G[:, i * 128:(i + 1) * 128].T
    ct = np.empty((128, 2, 128))
    for i in range(2):
        blk = C[:, i * 128:(i + 1) * 128].T                        # (128, 16)
        ct[:, i, :] = np.tile(blk, (1, 8))
    # halo stationary: shift group g lives at partitions 32g..32g+16,
    # weight W_g = Pm @ A^{L*(J-1-g)}; rows 32g+16..32g+32 stay zero.
    s01 = np.zeros((128, 512))
    for g in range(4):
        s01[32 * g:32 * g + 16, :] = (Pm @ Apow[L * (J - 1 - g)]).T
    T0 = ((T1 + J * L) + L - 1) // L * L
    return wT, ct, s01, T0


def _host_transient(meas, F, H, Q, R, T0):
    F64, H64 = F.astype(np.float64), H.astype(np.float64)
    Q64, R64 = Q.astype(np.float64), R.astype(np.float64)
    x = np.zeros(N)
    P = np.eye(N)
    out = np.empty((T0, N))
    for t in range(T0):
        x = F64 @ x
        P = F64 @ P @ F64.T + Q64
        z = meas[t, :, 0].astype(np.float64)
        S = H64 @ P @ H64.T + R64
        K = P @ H64.T @ np.linalg.inv(S)
        x = x + K @ (z - H64 @ x)
        P = (np.eye(N) - K @ H64) @ P
        out[t] = x
    return out


# ------------------------------------------------------------- device program
def _build_program(dtype_mode, out_mode):
    import concourse.bacc as bacc
    import concourse.bass as bass
    import concourse.tile as tile
    from concourse import mybir

    f32 = mybir.dt.float32
    cdt = {"bf16": mybir.dt.bfloat16,
           "fp32r": mybir.dt.float32r}.get(dtype_mode, f32)
    odt = mybir.dt.bfloat16 if out_mode == "bf16" else f32

    nc = bacc.Bacc("TRN2", target_bir_lowering=False, debug=False,
                   enable_asserts=False, num_devices=NCORES)

    zmat_d = nc.dram_tensor("zmat", [128, 2, KC], cdt, kind="ExternalInput").ap()
    wT_d = nc.dram_tensor("wT", [128, 2, 512], cdt, kind="ExternalInput").ap()
    cT_d = nc.dram_tensor("cT", [128, 2, 128], cdt, kind="ExternalInput").ap()
    s01_d = nc.dram_tensor("s01", [128, 512], cdt, kind="ExternalInput").ap()
    out_d = nc.dram_tensor("out", [128, 4, KB], odt, kind="ExternalOutput").ap()

    with tile.TileContext(nc, trace_sim=False) as tc:
        with (
            tc.tile_pool(name="const", bufs=1) as const,
            tc.tile_pool(name="psA", bufs=2, space=bass.MemorySpace.PSUM) as psA,
            tc.tile_pool(name="psC", bufs=6, space=bass.MemorySpace.PSUM) as psC,
        ):
            wt = const.tile([128, 2, 512], cdt, name="wt")
            ct = const.tile([128, 2, 128], cdt, name="ct")
            s01 = const.tile([128, 512], cdt, name="s01")
            scr = const.tile([128, 514], cdt, name="scr")
            # whole z input resident in SBUF (1 MB); each K-half is ONE
            # 128-descriptor DMA (3928 B rows) — queue throughput is
            # packet-count-bound (~135 pkts/us/queue), so minimize packets
            zfull = const.tile([128, 2, KC], cdt, name="zfull")
            # output singles (one per YGROUP) and per-stripe views
            ysbs = []
            yview = [None] * NS
            yoff = [0] * NS
            for gi, (i0, nst) in enumerate(YGROUPS):
                gw = sum(WIDTHS[i0:i0 + nst])
                yg = const.tile([128, 4, gw], odt, name=f"yg{gi}")
                ysbs.append(yg)
                off = 0
                for k in range(nst):
                    yview[i0 + k] = yg
                    yoff[i0 + k] = off
                    off += WIDTHS[i0 + k]
            # ust singles (one per stripe)
            usts = [const.tile([128, 512], cdt, name=f"ust{i}")
                    for i in range(NS)]

            # --- input DMA kicks, all up front; per-queue FIFO order paces
            # the stream in consumption order:
            #   sync:   zA.h0, wt.h0, zB1.h0
            #   scalar: zA.h1, s01, wt.h1, zB1.h1
            #   gpsimd: ct, zB2 (both halves — idle queue, rides out jitter)
            ZS = 1024  # pair-0 needs cols < 1024; land them first
            nc.sync.dma_start(zfull[:, 0, :ZS], zmat_d[:, 0, :ZS])
            nc.scalar.dma_start(zfull[:, 1, :ZS], zmat_d[:, 1, :ZS])
            nc.gpsimd.dma_start(ct[:], cT_d[:])
            ZS2 = 1524  # stripe-2 (and pass-A(2)) only needs cols < 1524
            nc.sync.dma_start(wt[:, 0], wT_d[:, 0])
            nc.scalar.dma_start(s01[:], s01_d[:])
            nc.scalar.dma_start(wt[:, 1], wT_d[:, 1])
            nc.sync.dma_start(zfull[:, 0, ZS:ZS2], zmat_d[:, 0, ZS:ZS2])
            nc.scalar.dma_start(zfull[:, 1, ZS:ZS2], zmat_d[:, 1, ZS:ZS2])
            nc.gpsimd.dma_start(zfull[:, 0, ZS2:], zmat_d[:, 0, ZS2:])
            nc.gpsimd.dma_start(zfull[:, 1, ZS2:], zmat_d[:, 1, ZS2:])
            # preload the activation table so stripe 0's scalar copies
            # don't eat the lazy ACT_TABLE_LOAD (scr read uninitialized
            # on purpose; results discarded)
            nc.scalar.copy(scr[0:32, 512:513], scr[0:32, 513:514])
            # HAM warm-up: dummy matmuls keep the PE busy through the
            # preamble + first-DMA window (into the psA pool so the psC
            # rotation is untouched)
            # HAM needs ~3.4us of SUSTAINED PE activity to unthrottle the
            # clock to 2.4 GHz, and any multi-us idle re-throttles it: keep
            # the PE busy from preamble until the z input lands
            pw = psA.tile([128, 512], f32, name="pw", tag="pu")
            for _ in range(8):
                nc.tensor.matmul(pw[:, :512], scr[:, 0:128], scr[:, 0:512],
                                 start=True, stop=True)
            # fine-grained warmup tail: rides out z-arrival jitter without
            # a multi-us PE idle (which would re-throttle the clock)
            for _ in range(4):
                nc.tensor.matmul(pw[:, :128], scr[:, 0:128], scr[:, 0:128],
                                 start=True, stop=True)

            def pass_a_pair(a, b):
                """U = C Z for two stripes, sharing each ct stationary."""
                pus = {i: psA.tile([128, 512], f32, name="pu", tag="pu")
                       for i in (a, b)}
                for i in (a, b):
                    for h in (0, 1):
                        s, w = STARTS[i], WIDTHS[i]
                        nc.tensor.matmul(pus[i][:, :w + 4], ct[:, h, :],
                                         zfull[:, h, s:s + w + 4],
                                         start=(h == 0), stop=(h == 1))
                # shifts: ust[32g+r, k] = U[r, k+g].  g0/g1 cast straight
                # out of PSUM; g2/g3 are 2x-rate bf16 cross-quadrant copies
                # from g0's rows (all PSUM quadrants hold identical U)
                for i in (a, b):
                    w = WIDTHS[i]
                    pu, ust = pus[i], usts[i]
                    nc.vector.tensor_copy(ust[0:32, :w + 4],
                                          pu[0:32, 0:w + 4])
                    nc.scalar.copy(ust[32:64, :w], pu[32:64, 1:w + 1])
                    nc.vector.tensor_copy(ust[64:96, :w], ust[0:32, 2:2 + w])
                    nc.vector.tensor_copy(ust[96:128, :w],
                                          ust[0:32, 3:3 + w])

            def evict(i, mt, py, to_dve, split=False):
                w = WIDTHS[i]
                ysb, yo = yview[i], yoff[i]
                if split:
                    # final eviction: halves on both engines so the last
                    # out-DMA kick fires sooner
                    hw = w // 2
                    nc.vector.tensor_copy(ysb[:, mt, yo:yo + hw], py[:, :hw])
                    nc.scalar.copy(ysb[:, mt, yo + hw:yo + w], py[:, hw:w])
                    return
                dst = ysb[:, mt, yo:yo + w]
                if to_dve:
                    nc.vector.tensor_copy(dst, py[:, :w])
                else:
                    nc.scalar.copy(dst, py[:, :w])

            def do_pair(a, b, mid_hook=None, last_pair=False,
                        carry=None, defer_last=False):
                """Z+halo for two stripes, mt-phase-major so every
                stationary (wt columns, s01 columns) is loaded once and
                consumed by both stripes' matmuls back to back."""
                def haloblock(mt, pys):
                    ms = slice(mt * 128, mt * 128 + 128)
                    for i in (a, b):
                        w = WIDTHS[i]
                        nc.tensor.matmul(pys[i][:, :w], s01[:, ms],
                                         usts[i][:, 0:w],
                                         start=False, stop=True)
                    last_phase = last_pair and mt == 3
                    evict(a, mt, pys[a], to_dve=(mt % 2 == 0),
                          split=last_phase)
                    evict(b, mt, pys[b],
                          to_dve=(mt % 2 == 1) or (last_pair and mt == 2),
                          split=last_phase)
                    kick_plane(a, mt)
                    kick_plane(b, mt)

                # software-pipelined: phase mt's halo+evict+kick emit after
                # phase mt+1's Z matmuls, so the halo's ust dependency has
                # an extra phase of slack and the next phase's weight loads
                # prefetch under the halo matmuls
                prev = None
                for mt in range(4):
                    ms = slice(mt * 128, mt * 128 + 128)
                    pys = {i: psC.tile([128, 512], f32, name="py", tag="py")
                           for i in (a, b)}
                    clist = ([(0, 64)] if mt == 0 else
                             [(0, 128)] if mt == 1 else
                             [(0, 128), (1, 64)] if mt == 2 else
                             [(0, 128), (1, 128)])
                    for j, (h, p) in enumerate(clist):
                        for i in (a, b):
                            s, w = STARTS[i], WIDTHS[i]
                            nc.tensor.matmul(pys[i][:, :w], wt[0:p, h, ms],
                                             zfull[0:p, h, s + J:s + J + w],
                                             start=(j == 0), stop=False)
                    if mt == 0 and carry is not None:
                        carry()   # previous pair's deferred mt3 halo
                    if prev is not None:
                        haloblock(prev[0], prev[1])
                        if prev[0] == 1 and mid_hook is not None:
                            mid_hook()
                    prev = (mt, pys)
                if defer_last:
                    pys3 = prev[1]
                    return lambda: haloblock(3, pys3)
                haloblock(3, prev[1])
                return None

            # queue map per stripe: balance ~1MB of output per fast queue;
            # the scalar-engine kicks are few and late enough not to stall
            # the ACT copy pipeline
            OUT_ENG = {0: ("sync", "sync"), 1: ("scalar", "scalar"),
                       2: ("sync", "sync"), 3: ("sync", "scalar")}

            def kick_plane(it, mt):
                """DMA plane `mt` of stripe `it`'s output."""
                s, gw = STARTS[it], WIDTHS[it]
                yg = ysbs[it]
                eng = getattr(nc, OUT_ENG[it][0 if mt < 2 else 1])
                eng.dma_start(out_d[:, mt, s:s + gw], yg[:, mt, :])

            pass_a_pair(0, 1)
            c0 = do_pair(0, 1, mid_hook=lambda: pass_a_pair(2, 3),
                         defer_last=True)
            do_pair(2, 3, carry=c0, last_pair=True)
    nc.compile()
    return nc


# ------------------------------------------------------------------ interface
def _np_dt(dtype_mode):
    if dtype_mode == "bf16":
        import ml_dtypes
        return ml_dtypes.bfloat16
    return np.float32


def _prepare(measurements, F, H, Q, R, dtype_mode):
    wT, ct, s01, T0 = _build_weights(F, H, Q, R)
    np_dt = _np_dt(dtype_mode)
    wT = np.ascontiguousarray(wT.astype(np.float32)).astype(np_dt)
    ct = np.ascontiguousarray(ct.astype(np.float32)).astype(np_dt)
    s01 = np.ascontiguousarray(s01.astype(np.float32)).astype(np_dt)

    meas_pad = np.zeros((TTOT, M), np.float32)
    meas_pad[:T] = measurements[:, :, 0]
    # blocks[k, i, p]: block k, K-half i, component p (z comp (i*128+p))
    blocks = meas_pad.reshape(TTOT // L, 2, 128)

    in_maps = []
    for c in range(NCORES):
        k0 = c * KB
        zc = np.zeros((128, 2, KC), np.float32)
        lo = max(0, k0 - J)
        src = blocks[lo:k0 + KB].transpose(2, 1, 0)   # (128, 2, ncols)
        zc[:, :, J - (k0 - lo):] = src
        in_maps.append({"zmat": np.ascontiguousarray(zc).astype(np_dt),
                        "wT": wT, "cT": ct, "s01": s01})
    return in_maps, T0


def _assemble(results, meas, F, H, Q, R, T0):
    chunks = []
    for c in range(NCORES):
        o = np.asarray(results[c]["out"], dtype=np.float32)  # (128,4,KB)
        Y = o.transpose(1, 0, 2).reshape(512, KB)
        chunks.append(np.ascontiguousarray(Y.T).reshape(KB * L, N))
    full = np.concatenate(chunks, axis=0)[:T]
    full[:T0] = _host_transient(meas, F, H, Q, R, T0).astype(np.float32)
    return np.ascontiguousarray(full).reshape(T, N, 1).astype(np.float32)


def run(measurements, F, H, Q, R, trace=False):
    """Returns (output, BassKernelResults)."""
    from concourse.bass_utils import run_bass_kernel_spmd

    key = (DTYPE_MODE, OUT_MODE)
    if _cache.get("key") != key:
        _cache["nc"] = _build_program(*key)
        _cache["key"] = key
    nc = _cache["nc"]
    in_maps, T0 = _prepare(measurements, F, H, Q, R, DTYPE_MODE)
    res = run_bass_kernel_spmd(nc, in_maps, core_ids=list(range(NCORES)),
                               trace=trace)
    out = _assemble(res.results, measurements, F, H, Q, R, T0)
    return out, res


def kernel(measurements, F, H, Q, R):
    measurements = np.asarray(measurements, dtype=np.float32)
    F = np.asarray(F, dtype=np.float32)
    H = np.asarray(H, dtype=np.float32)
    Q = np.asarray(Q, dtype=np.float32)
    R = np.asarray(R, dtype=np.float32)
    out, _ = run(measurements, F, H, Q, R, trace=False)
    return out


# revision 55
# speedup vs baseline: 1.1370x; 1.1370x over previous
"""Trainium2 Bass kernel for nn_BackpropKalmanFilter (v4).

After the Riccati recursion converges the filter is the LTI recursion
    x_t = A x_{t-1} + K z_t,   A = (I - K H) F   (rho(A) ~ 0.96)
Blocks of L=32 steps, U_b = C Z_b (end-of-block response), halo over J=4
previous blocks; the pre-convergence transient is recomputed on the host.

Structure (vs v2; each driven by a trace finding):
  - 4 stripes [508, 508, 504, 440] processed as two PAIRS, mt-phase-major:
    per phase both stripes' Z matmuls share each stationary; each phase's
    halo+eviction is software-pipelined one phase later (emitted after the
    NEXT phase's Z matmuls, carried across the pair boundary too) so halo
    dependencies have extra slack and the next weight loads prefetch under
    the halo matmuls.
  - z input resident in SBUF, loaded as few LONG-descriptor DMAs (3928 B
    rows, split only where early completion matters): DMA queues are
    packet-rate bound (~135 pkts/us/queue), not byte bound.
  - All SBUF tiles are singles (no pool rotation); per-queue FIFO order
    paces input arrival in consumption order; outputs kick per stripe,
    sync queue for mt01, scalar/sync mix for mt23 (never a scalar-engine
    kick mid-compute: it stalls the ACT copy pipeline -> PSUM WAR -> PE).
  - 8+8 HAM warmup matmuls (coarse+fine tail) keep the PE busy until z
    lands: HAM needs
    ~3.4us of sustained activity to unthrottle 1.2->2.4 GHz and any
    multi-us idle re-throttles it.
All operands bf16, accumulation f32, output bf16.
"""
import os
import sys

import numpy as np

sys.path.insert(0, "/opt/trn_rl_repo")
sys.path.insert(0, "/root/.axon_site")
sys.path.insert(0, "/root/.axon_site/_ro/pypackages")

N, M = 16, 8          # state / measurement dims
T = 500_000
L = 32                # block length
J = 4                 # halo blocks (D = J*L = 128 decay length)
NCORES = 8
KB = 1960                         # blocks per core
TTOT = NCORES * KB * L            # 501760 padded steps
KC = KB + J                       # columns incl. halo
DTYPE_MODE = os.environ.get("KAL_DTYPE", "bf16")   # bf16 | fp32r | fp32
OUT_MODE = os.environ.get("KAL_OUTDT", "bf16")     # bf16 | fp32

WIDTHS = [508, 508, 504, 440]
STARTS = [0]
for _w in WIDTHS[:-1]:
    STARTS.append(STARTS[-1] + _w)
NS = len(WIDTHS)
# output groups: (first stripe idx, n stripes)
YGROUPS = [(i, 1) for i in range(NS)]

_cache = {}


# ----------------------------------------------------------------- host math
def _riccati(F, H, Q, R):
    F64, H64 = F.astype(np.float64), H.astype(np.float64)
    Q64, R64 = Q.astype(np.float64), R.astype(np.float64)
    P = np.eye(N)
    prevK = None
    T1 = None
    for t in range(2048):
        P = F64 @ P @ F64.T + Q64
        S = H64 @ P @ H64.T + R64
        K = P @ H64.T @ np.linalg.inv(S)
        P = (np.eye(N) - K @ H64) @ P
        if prevK is not None and T1 is None and np.abs(K - prevK).max() < 1e-13:
            T1 = t
        prevK = K
    assert T1 is not None
    A = (np.eye(N) - K @ H64) @ F64
    return K, A, T1


def _build_weights(F, H, Q, R):
    """Returns (wT [128,2,512], ct [128,2,128], s01 [128,512], T0) in f64."""
    K_ss, A, T1 = _riccati(F, H, Q, R)
    npow = L * J + 2
    Apow = np.empty((npow, N, N))
    Apow[0] = np.eye(N)
    for i in range(1, npow):
        Apow[i] = Apow[i - 1] @ A
    AK = Apow @ K_ss                                   # A^d K  (16 x 8)

    C = np.concatenate([AK[L - 1 - j] for j in range(L)], axis=1)  # (16, 256)
    G = np.zeros((N * L, M * L))
    for i in range(L):
        for j in range(i + 1):
            G[i * N:(i + 1) * N, j * M:(j + 1) * M] = AK[i - j]
    Pm = np.concatenate([Apow[i + 1] for i in range(L)], axis=0)   # (512, 16)

    wT = np.empty((128, 2, 512))
    for i in range(2):
        wT[:, i, :] = G[:, i * 128:(i + 1) * 128].T
    ct = np.empty((128, 2, 128))
    for i in range(2):
        blk = C[:, i * 128:(i + 1) * 128].T                        # (128, 16)
        ct[:, i, :] = np.tile(blk, (1, 8))
    # halo stationary: shift group g lives at partitions 32g..32g+16,
    # weight W_g = Pm @ A^{L*(J-1-g)}; rows 32g+16..32g+32 stay zero.
    s01 = np.zeros((128, 512))
    for g in range(4):
        s01[32 * g:32 * g + 16, :] = (Pm @ Apow[L * (J - 1 - g)]).T
    T0 = ((T1 + J * L) + L - 1) // L * L
    return wT, ct, s01, T0


def _host_transient(meas, F, H, Q, R, T0):
    F64, H64 = F.astype(np.float64), H.astype(np.float64)
    Q64, R64 = Q.astype(np.float64), R.astype(np.float64)
    x = np.zeros(N)
    P = np.eye(N)
    out = np.empty((T0, N))
    for t in range(T0):
        x = F64 @ x
        P = F64 @ P @ F64.T + Q64
        z = meas[t, :, 0].astype(np.float64)
        S = H64 @ P @ H64.T + R64
        K = P @ H64.T @ np.linalg.inv(S)
        x = x + K @ (z - H64 @ x)
        P = (np.eye(N) - K @ H64) @ P
        out[t] = x
    return out


# ------------------------------------------------------------- device program
def _build_program(dtype_mode, out_mode):
    import concourse.bacc as bacc
    import concourse.bass as bass
    import concourse.tile as tile
    from concourse import mybir

    f32 = mybir.dt.float32
    cdt = {"bf16": mybir.dt.bfloat16,
           "fp32r": mybir.dt.float32r}.get(dtype_mode, f32)
    odt = mybir.dt.bfloat16 if out_mode == "bf16" else f32

    nc = bacc.Bacc("TRN2", target_bir_lowering=False, debug=False,
                   enable_asserts=False, num_devices=NCORES)

    zmat_d = nc.dram_tensor("zmat", [128, 2, KC], cdt, kind="ExternalInput").ap()
    wT_d = nc.dram_tensor("wT", [128, 2, 512], cdt, kind="ExternalInput").ap()
    cT_d = nc.dram_tensor("cT", [128, 2, 128], cdt, kind="ExternalInput").ap()
    s01_d = nc.dram_tensor("s01", [128, 512], cdt, kind="ExternalInput").ap()
    out_d = nc.dram_tensor("out", [128, 4, KB], odt, kind="ExternalOutput").ap()

    with tile.TileContext(nc, trace_sim=False) as tc:
        with (
            tc.tile_pool(name="const", bufs=1) as const,
            tc.tile_pool(name="psA", bufs=2, space=bass.MemorySpace.PSUM) as psA,
            tc.tile_pool(name="psC", bufs=6, space=bass.MemorySpace.PSUM) as psC,
        ):
            wt = const.tile([128, 2, 512], cdt, name="wt")
            ct = const.tile([128, 2, 128], cdt, name="ct")
            s01 = const.tile([128, 512], cdt, name="s01")
            scr = const.tile([128, 514], cdt, name="scr")
            # whole z input resident in SBUF (1 MB); each K-half is ONE
            # 128-descriptor DMA (3928 B rows) — queue throughput is
            # packet-count-bound (~135 pkts/us/queue), so minimize packets
            zfull = const.tile([128, 2, KC], cdt, name="zfull")
            # output singles (one per YGROUP) and per-stripe views
            ysbs = []
            yview = [None] * NS
            yoff = [0] * NS
            for gi, (i0, nst) in enumerate(YGROUPS):
                gw = sum(WIDTHS[i0:i0 + nst])
                yg = const.tile([128, 4, gw], odt, name=f"yg{gi}")
                ysbs.append(yg)
                off = 0
                for k in range(nst):
                    yview[i0 + k] = yg
                    yoff[i0 + k] = off
                    off += WIDTHS[i0 + k]
            # ust singles (one per stripe)
            usts = [const.tile([128, 512], cdt, name=f"ust{i}")
                    for i in range(NS)]

            # --- input DMA kicks, all up front; per-queue FIFO order paces
            # the stream in consumption order:
            #   sync:   zA.h0, wt.h0, zB1.h0
            #   scalar: zA.h1, s01, wt.h1, zB1.h1
            #   gpsimd: ct, zB2 (both halves — idle queue, rides out jitter)
            ZS = 1024  # pair-0 needs cols < 1024; land them first
            nc.sync.dma_start(zfull[:, 0, :ZS], zmat_d[:, 0, :ZS])
            nc.scalar.dma_start(zfull[:, 1, :ZS], zmat_d[:, 1, :ZS])
            nc.gpsimd.dma_start(ct[:], cT_d[:])
            ZS2 = 1524  # stripe-2 (and pass-A(2)) only needs cols < 1524
            nc.sync.dma_start(wt[:, 0], wT_d[:, 0])
            nc.scalar.dma_start(s01[:], s01_d[:])
            nc.scalar.dma_start(wt[:, 1], wT_d[:, 1])
            nc.sync.dma_start(zfull[:, 0, ZS:ZS2], zmat_d[:, 0, ZS:ZS2])
            nc.scalar.dma_start(zfull[:, 1, ZS:ZS2], zmat_d[:, 1, ZS:ZS2])
            nc.gpsimd.dma_start(zfull[:, 0, ZS2:], zmat_d[:, 0, ZS2:])
            nc.gpsimd.dma_start(zfull[:, 1, ZS2:], zmat_d[:, 1, ZS2:])
            # preload the activation table so stripe 0's scalar copies
            # don't eat the lazy ACT_TABLE_LOAD (scr read uninitialized
            # on purpose; results discarded)
            nc.scalar.copy(scr[0:32, 512:513], scr[0:32, 513:514])
            # HAM warm-up: dummy matmuls keep the PE busy through the
            # preamble + first-DMA window (into the psA pool so the psC
            # rotation is untouched)
            # HAM needs ~3.4us of SUSTAINED PE activity to unthrottle the
            # clock to 2.4 GHz, and any multi-us idle re-throttles it: keep
            # the PE busy from preamble until the z input lands
            pw = psA.tile([128, 512], f32, name="pw", tag="pu")
            for _ in range(8):
                nc.tensor.matmul(pw[:, :512], scr[:, 0:128], scr[:, 0:512],
                                 start=True, stop=True)
            # fine-grained warmup tail: rides out z-arrival jitter without
            # a multi-us PE idle (which would re-throttle the clock)
            for _ in range(8):
                nc.tensor.matmul(pw[:, :128], scr[:, 0:128], scr[:, 0:128],
                                 start=True, stop=True)

            def pass_a_pair(a, b):
                """U = C Z for two stripes, sharing each ct stationary."""
                pus = {i: psA.tile([128, 512], f32, name="pu", tag="pu")
                       for i in (a, b)}
                for i in (a, b):
                    for h in (0, 1):
                        s, w = STARTS[i], WIDTHS[i]
                        nc.tensor.matmul(pus[i][:, :w + 4], ct[:, h, :],
                                         zfull[:, h, s:s + w + 4],
                                         start=(h == 0), stop=(h == 1))
                # shifts: ust[32g+r, k] = U[r, k+g].  g0/g1 cast straight
                # out of PSUM; g2/g3 are 2x-rate bf16 cross-quadrant copies
                # from g0's rows (all PSUM quadrants hold identical U)
                for i in (a, b):
                    w = WIDTHS[i]
                    pu, ust = pus[i], usts[i]
                    nc.vector.tensor_copy(ust[0:32, :w + 4],
                                          pu[0:32, 0:w + 4])
                    nc.scalar.copy(ust[32:64, :w], pu[32:64, 1:w + 1])
                    nc.vector.tensor_copy(ust[64:96, :w], ust[0:32, 2:2 + w])
                    nc.vector.tensor_copy(ust[96:128, :w],
                                          ust[0:32, 3:3 + w])

            def evict(i, mt, py, to_dve, split=False):
                w = WIDTHS[i]
                ysb, yo = yview[i], yoff[i]
                if split:
                    # final eviction: halves on both engines so the last
                    # out-DMA kick fires sooner
                    hw = w // 2
                    nc.vector.tensor_copy(ysb[:, mt, yo:yo + hw], py[:, :hw])
                    nc.scalar.copy(ysb[:, mt, yo + hw:yo + w], py[:, hw:w])
                    return
                dst = ysb[:, mt, yo:yo + w]
                if to_dve:
                    nc.vector.tensor_copy(dst, py[:, :w])
                else:
                    nc.scalar.copy(dst, py[:, :w])

            def do_pair(a, b, mid_hook=None, last_pair=False,
                        carry=None, defer_last=False):
                """Z+halo for two stripes, mt-phase-major so every
                stationary (wt columns, s01 columns) is loaded once and
                consumed by both stripes' matmuls back to back."""
                def haloblock(mt, pys):
                    ms = slice(mt * 128, mt * 128 + 128)
                    for i in (a, b):
                        w = WIDTHS[i]
                        nc.tensor.matmul(pys[i][:, :w], s01[:, ms],
                                         usts[i][:, 0:w],
                                         start=False, stop=True)
                    last_phase = last_pair and mt == 3
                    evict(a, mt, pys[a], to_dve=(mt % 2 == 0),
                          split=last_phase)
                    evict(b, mt, pys[b],
                          to_dve=(mt % 2 == 1) or (last_pair and mt == 2),
                          split=last_phase)
                    kick_plane(a, mt)
                    kick_plane(b, mt)

                # software-pipelined: phase mt's halo+evict+kick emit after
                # phase mt+1's Z matmuls, so the halo's ust dependency has
                # an extra phase of slack and the next phase's weight loads
                # prefetch under the halo matmuls
                prev = None
                for mt in range(4):
                    ms = slice(mt * 128, mt * 128 + 128)
                    pys = {i: psC.tile([128, 512], f32, name="py", tag="py")
                           for i in (a, b)}
                    clist = ([(0, 64)] if mt == 0 else
                             [(0, 128)] if mt == 1 else
                             [(0, 128), (1, 64)] if mt == 2 else
                             [(0, 128), (1, 128)])
                    for j, (h, p) in enumerate(clist):
                        for i in (a, b):
                            s, w = STARTS[i], WIDTHS[i]
                            nc.tensor.matmul(pys[i][:, :w], wt[0:p, h, ms],
                                             zfull[0:p, h, s + J:s + J + w],
                                             start=(j == 0), stop=False)
                    if mt == 0 and carry is not None:
                        carry()   # previous pair's deferred mt3 halo
                    if prev is not None:
                        haloblock(prev[0], prev[1])
                        if prev[0] == 1 and mid_hook is not None:
                            mid_hook()
                    prev = (mt, pys)
                if defer_last:
                    pys3 = prev[1]
                    return lambda: haloblock(3, pys3)
                haloblock(3, prev[1])
                return None

            # queue map per stripe: balance ~1MB of output per fast queue;
            # the scalar-engine kicks are few and late enough not to stall
            # the ACT copy pipeline
            OUT_ENG = {0: ("sync", "sync"), 1: ("scalar", "scalar"),
                       2: ("sync", "sync"), 3: ("sync", "scalar")}

            def kick_plane(it, mt):
                """DMA plane `mt` of stripe `it`'s output."""
                s, gw = STARTS[it], WIDTHS[it]
                yg = ysbs[it]
                eng = getattr(nc, OUT_ENG[it][0 if mt < 2 else 1])
                eng.dma_start(out_d[:, mt, s:s + gw], yg[:, mt, :])

            pass_a_pair(0, 1)
            c0 = do_pair(0, 1, mid_hook=lambda: pass_a_pair(2, 3),
                         defer_last=True)
            do_pair(2, 3, carry=c0, last_pair=True)
    nc.compile()
    return nc


# ------------------------------------------------------------------ interface
def _np_dt(dtype_mode):
    if dtype_mode == "bf16":
        import ml_dtypes
        return ml_dtypes.bfloat16
    return np.float32


def _prepare(measurements, F, H, Q, R, dtype_mode):
    wT, ct, s01, T0 = _build_weights(F, H, Q, R)
    np_dt = _np_dt(dtype_mode)
    wT = np.ascontiguousarray(wT.astype(np.float32)).astype(np_dt)
    ct = np.ascontiguousarray(ct.astype(np.float32)).astype(np_dt)
    s01 = np.ascontiguousarray(s01.astype(np.float32)).astype(np_dt)

    meas_pad = np.zeros((TTOT, M), np.float32)
    meas_pad[:T] = measurements[:, :, 0]
    # blocks[k, i, p]: block k, K-half i, component p (z comp (i*128+p))
    blocks = meas_pad.reshape(TTOT // L, 2, 128)

    in_maps = []
    for c in range(NCORES):
        k0 = c * KB
        zc = np.zeros((128, 2, KC), np.float32)
        lo = max(0, k0 - J)
        src = blocks[lo:k0 + KB].transpose(2, 1, 0)   # (128, 2, ncols)
        zc[:, :, J - (k0 - lo):] = src
        in_maps.append({"zmat": np.ascontiguousarray(zc).astype(np_dt),
                        "wT": wT, "cT": ct, "s01": s01})
    return in_maps, T0


def _assemble(results, meas, F, H, Q, R, T0):
    chunks = []
    for c in range(NCORES):
        o = np.asarray(results[c]["out"], dtype=np.float32)  # (128,4,KB)
        Y = o.transpose(1, 0, 2).reshape(512, KB)
        chunks.append(np.ascontiguousarray(Y.T).reshape(KB * L, N))
    full = np.concatenate(chunks, axis=0)[:T]
    full[:T0] = _host_transient(meas, F, H, Q, R, T0).astype(np.float32)
    return np.ascontiguousarray(full).reshape(T, N, 1).astype(np.float32)


def run(measurements, F, H, Q, R, trace=False):
    """Returns (output, BassKernelResults)."""
    from concourse.bass_utils import run_bass_kernel_spmd

    key = (DTYPE_MODE, OUT_MODE)
    if _cache.get("key") != key:
        _cache["nc"] = _build_program(*key)
        _cache["key"] = key
    nc = _cache["nc"]
    in_maps, T0 = _prepare(measurements, F, H, Q, R, DTYPE_MODE)
    res = run_bass_kernel_spmd(nc, in_maps, core_ids=list(range(NCORES)),
                               trace=trace)
    out = _assemble(res.results, measurements, F, H, Q, R, T0)
    return out, res


def kernel(measurements, F, H, Q, R):
    measurements = np.asarray(measurements, dtype=np.float32)
    F = np.asarray(F, dtype=np.float32)
    H = np.asarray(H, dtype=np.float32)
    Q = np.asarray(Q, dtype=np.float32)
    R = np.asarray(R, dtype=np.float32)
    out, _ = run(measurements, F, H, Q, R, trace=False)
    return out
